# revision 2
# baseline (speedup 1.0000x reference)
"""Viterbi CRF decode kernel for Trainium2, data-parallel over batch on 8 cores.

Device computes the forward Viterbi max-plus scan, storing the full fp32
partition history; the host recomputes backpointers lazily along the decoded
path (bit-exact vs the jax reference). See _host_backtrack.

Per-core structure (16 batches, partitions p=(b2,tag), pairs c in 0..7):
  cur(t) lives in PSUM, built as  base(f+trans) + replicated part(t-1):
    - base chunks prebuilt on GPSIMD (bulk), injected SBUF->PSUM by ACT copy
      (prefetched `depth` steps ahead, off the critical chain)
    - part(t-1) replication: DVE builds diag(part) = part-bcast * diag-mask,
      one PE matmul with block-ones lhsT accumulates it onto the base
      (start=False): PSUM = (f + trans) + part, the reference's add order
    - DVE tensor_reduce(max) over i -> part(t) -> part_hist
  Two independent chain groups (4 pairs each) pipeline across DVE/PE/ACT.
"""

import os
import sys

sys.path.insert(0, "/opt/trn_rl_repo")

import numpy as np

import concourse.bass as bass
import concourse.mybir as mybir
import concourse.tile as tile
from concourse.vector_clock import ScopedClock

B, S, T = 128, 512, 64
START_TAG, STOP_TAG = T - 2, T - 1
N_CORES = 8
BPC = B // N_CORES          # batches per core = 16
NCH = BPC // 2              # chain pairs per core = 8

_F32 = mybir.dt.float32

# tuned schedule parameters
CFG = dict(G=2, binj="act", diag="dve", tchunk=16, depth=3)


def _patch_tile_drain():
    """walrus in this toolchain rejects >1-2 sem waits on one CTRL
    instruction; split the TileContext tail-drain waits one-per-nop."""

    def _patched(self, tick_clock, wait_clock):
        carrier = self.nc.sync.nop()
        wait_clock.add_sem_waits(
            carrier.ins, ScopedClock({None: tick_clock.global_clock})
        )
        si = carrier.ins.sync_info
        waits = list(si.on_wait) if si and si.on_wait else []
        upds = list(si.on_update) if si and si.on_update else []
        if len(waits) > 1:
            carrier.ins.sync_info = mybir.SyncInfo(on_wait=[waits[0]], on_update=upds)
            for w in waits[1:]:
                n = self.nc.sync.nop()
                n.ins.sync_info = mybir.SyncInfo(on_wait=[w], on_update=[])
        self.nc.sync.drain()
        self.nc.all_engine_barrier()
        assert self.sems is not None
        popped = self.nc._tile_sem_poison_stack.pop()
        assert popped is self._sem_poison
        self.nc.all_engine_barrier()

    tile.TileContext._drain_and_barrier = _patched

    orig_add = tile.TileContext._add_instruction

    def _add_split(self, inst):
        si = getattr(inst, "sync_info", None)
        waits = list(si.on_wait) if si and si.on_wait else []
        lim = 1
        if len(waits) > lim:
            head, rest = waits[:lim], waits[lim:]
            for w in rest:
                carrier = mybir.InstNoOp(
                    name=self.nc.get_next_instruction_name(),
                    sync_info=mybir.SyncInfo(on_wait=[w], on_update=[]),
                    bass_nofuse=True,
                    engine=inst.engine,
                )
                orig_add(self, carrier)
            inst.sync_info = mybir.SyncInfo(
                on_wait=head, on_update=list(si.on_update or [])
            )
        orig_add(self, inst)

    tile.TileContext._add_instruction = _add_split


_patch_tile_drain()


def build_forward_kernel(cfg=None):
    """One NeuronCore's forward-scan bass module (config-C structure)."""
    cfg = dict(CFG, **(cfg or {}))
    G = cfg["G"]
    binj = cfg["binj"]
    diag_eng = cfg["diag"]
    if isinstance(diag_eng, str):
        diag_eng = [diag_eng] * G
    TCHUNK = cfg["tchunk"]
    D = cfg["depth"]
    PPG = NCH // G

    nc = bass.Bass()
    featsT = nc.declare_dram_parameter("featsT", [128, S, NCH], _F32, isOutput=False)
    transT = nc.declare_dram_parameter("transT", [T, T], _F32, isOutput=False)
    ident = nc.declare_dram_parameter("ident", [128, 128], _F32, isOutput=False)
    bones = nc.declare_dram_parameter("bones", [128, 128], _F32, isOutput=False)
    dmask = nc.declare_dram_parameter("dmask", [128, T], _F32, isOutput=False)
    parts = nc.declare_dram_parameter("parts", [128, S * NCH], _F32, isOutput=True)

    nchunks = (S + TCHUNK - 1) // TCHUNK

    with tile.TileContext(nc) as tc:
        with (
            tc.tile_pool(name="const", bufs=1) as constp,
            tc.tile_pool(name="hist", bufs=1) as histp,
            tc.tile_pool(name="ft", bufs=1) as ftp,
            tc.tile_pool(name="base", bufs=2) as basep,
            tc.tile_pool(name="work", bufs=1) as workp,
            tc.tile_pool(name="psum", bufs=1, space="PSUM") as psump,
        ):
            transRep = constp.tile([128, T], _F32, tag="transRep")
            nc.sync.dma_start(transRep[0:64, :], transT[:, :])
            nc.sync.dma_start(transRep[64:128, :], transT[:, :])
            identity = constp.tile([128, 128], _F32, tag="identity")
            nc.sync.dma_start(identity[:], ident[:])
            bones_sb = constp.tile([128, 128], _F32, tag="bones")
            nc.sync.dma_start(bones_sb[:], bones[:])
            dmask_sb = constp.tile([128, T], _F32, tag="dmask")
            nc.sync.dma_start(dmask_sb[:], dmask[:])

            part_hist = histp.tile([128, S * NCH], _F32, tag="part_hist")
            ft_all = ftp.tile([128, S, NCH], _F32, tag="ft")
            nc.sync.dma_start(ft_all[:], featsT[:])

            base_tiles = []

            def build_base(n):
                t0 = n * TCHUNK
                tn = min(TCHUNK, S - t0)
                ft = ft_all[:, t0:t0 + tn, :].rearrange("p s c -> p (s c)")
                bt = basep.tile([128, TCHUNK * NCH, T], _F32, tag="base")
                in0 = ft.unsqueeze(2).broadcast_to([128, tn * NCH, T])
                in1 = transRep[:].unsqueeze(1).broadcast_to([128, tn * NCH, T])
                nc.gpsimd.tensor_tensor(
                    bt[:, 0:tn * NCH, :], in0, in1, mybir.AluOpType.add
                )
                return bt

            base_tiles.append(build_base(0))
            if nchunks > 1:
                base_tiles.append(build_base(1))

            scr2 = [
                workp.tile([128, PPG, T], _F32, name=f"scr2_{g}", tag=f"scr2_{g}")
                for g in range(G)
            ]
            cur = [
                [
                    psump.tile([128, PPG, T], _F32, name=f"cur{g}_{d}",
                               tag=f"cur{g}_{d}")
                    for d in range(D)
                ]
                for g in range(G)
            ]

            def inject_base(g, t):
                c0 = g * PPG
                bt = base_tiles[t // TCHUNK]
                trel = t % TCHUNK
                src = bt[:, trel * NCH + c0:trel * NCH + c0 + PPG, :]
                dst = cur[g][t % D]
                if binj == "act":
                    nc.scalar.copy(dst[:], src)
                else:  # 'pe'
                    nc.tensor.matmul(
                        dst[:].rearrange("p c i -> p (c i)"),
                        identity[:],
                        src.rearrange("p c i -> p (c i)"),
                        start=True,
                        stop=False,
                        skip_group_check=True,
                    )

            def accum_part(g, t):
                c0 = g * PPG
                col = (t - 1) * NCH + c0
                np_view = part_hist[:, col:col + PPG]
                in0 = np_view.unsqueeze(2).broadcast_to([128, PPG, T])
                in1 = dmask_sb[:].unsqueeze(1).broadcast_to([128, PPG, T])
                eng = nc.vector if diag_eng[g] == "dve" else nc.gpsimd
                eng.tensor_tensor(scr2[g][:], in0, in1, mybir.AluOpType.mult)
                nc.tensor.matmul(
                    cur[g][t % D][:].rearrange("p c i -> p (c i)"),
                    bones_sb[:],
                    scr2[g][:].rearrange("p c i -> p (c i)"),
                    start=False,
                    stop=True,
                    skip_group_check=True,
                )

            nc.vector.tensor_scalar_add(
                part_hist[:, 0:NCH],
                ft_all[:, 0, :],
                transRep[:, START_TAG:START_TAG + 1],
            )

            for t in range(1, min(D + 1, S)):
                for g in range(G):
                    inject_base(g, t)

            for t in range(1, S):
                if t % TCHUNK == 0:
                    n = t // TCHUNK
                    if n + 1 < nchunks and n >= 1:
                        base_tiles.append(build_base(n + 1))
                for g in range(G):
                    c0 = g * PPG
                    accum_part(g, t)
                    nc.vector.tensor_reduce(
                        part_hist[:, t * NCH + c0:t * NCH + c0 + PPG],
                        cur[g][t % D][:],
                        axis=mybir.AxisListType.X,
                        op=mybir.AluOpType.max,
                    )
                    if t + D < S:
                        inject_base(g, t + D)

            nc.sync.dma_start(parts[:], part_hist[:])

    return nc


_FWD_CACHE = {}
LAST_EXEC_NS = None


def _sim_exec_ns(nc):
    """Cost-model (TimelineSim) predicted per-core duration in ns."""
    try:
        from concourse.timeline_sim import TimelineSim

        return int(TimelineSim(nc, no_exec=True).simulate())
    except Exception:
        return None


def _forward_on_device(feats_np, trans_np):
    """Run the forward scan on 8 cores. Returns part_hist [S, B, T] f32."""
    global LAST_EXEC_NS
    from concourse.bass_utils import run_bass_kernel_spmd

    if "nc" not in _FWD_CACHE:
        _FWD_CACHE["nc"] = build_forward_kernel()
    nc = _FWD_CACHE["nc"]

    if "sim_ns" not in _FWD_CACHE:
        _FWD_CACHE["sim_ns"] = _sim_exec_ns(nc)

    transT = np.ascontiguousarray(trans_np.T)
    ident = np.eye(128, dtype=np.float32)
    bones = np.zeros((128, 128), dtype=np.float32)
    bones[0:64, 0:64] = 1.0
    bones[64:128, 64:128] = 1.0
    dmask = np.zeros((128, T), dtype=np.float32)
    for u in range(T):
        dmask[u, u] = 1.0
        dmask[64 + u, u] = 1.0

    in_maps = []
    for k in range(N_CORES):
        shard = feats_np[k * BPC:(k + 1) * BPC]          # (16, S, T)
        # featsT[b2*64 + j, s, c] = shard[b2*8 + c, s, j]
        ft = np.ascontiguousarray(
            shard.reshape(2, NCH, S, T).transpose(0, 3, 2, 1).reshape(128, S, NCH)
        )
        in_maps.append(
            {"featsT": ft, "transT": transT, "ident": ident,
             "bones": bones, "dmask": dmask}
        )

    trace = bool(os.environ.get("CRF_TRACE"))
    res = run_bass_kernel_spmd(nc, in_maps, list(range(N_CORES)), trace=trace)
    if res.exec_time_ns is not None:
        LAST_EXEC_NS = res.exec_time_ns
    elif _FWD_CACHE["sim_ns"] is not None:
        LAST_EXEC_NS = _FWD_CACHE["sim_ns"]

    part = np.empty((S, B, T), dtype=np.float32)
    for k in range(N_CORES):
        p = res.results[k]["parts"].reshape(128, S, NCH)  # [(b2,j), t, c]
        p = p.reshape(2, T, S, NCH)                       # [b2, j, t, c]
        part[:, k * BPC:(k + 1) * BPC, :] = (
            p.transpose(2, 0, 3, 1).reshape(S, BPC, T)
        )
    return part


def _host_backtrack(part, feats, mask, trans):
    """Backpointer recompute + backtrack, bit-exact vs the jax reference."""
    f32 = np.float32
    lengths = mask.astype(np.int64).sum(axis=1)          # (B,)
    last_pos = lengths - 1
    bidx = np.arange(B)

    last_partition = part[last_pos, bidx, :]             # (B, T)
    last_vals = last_partition + trans[:, STOP_TAG][None, :].astype(f32)
    pointer0 = np.argmax(last_vals, axis=1).astype(np.int32)

    decode = np.zeros((S, B), dtype=np.int32)
    decode[S - 1] = pointer0
    ptr = pointer0
    trans_T = np.ascontiguousarray(trans.T)              # trans_T[j, i] = trans[i, j]
    for t in range(S - 2, -1, -1):
        jstar = ptr                                       # decode[t+1]
        fcol = feats[bidx, t + 1, jstar].astype(f32)      # (B,)
        cur = (fcol[:, None] + trans_T[jstar]) + part[t]  # (B, T) f32
        bp_val = np.argmax(cur, axis=1).astype(np.int32)
        new_ptr = np.where(
            t == last_pos, pointer0,
            np.where(t >= lengths, 0, bp_val)
        ).astype(np.int32)
        decode[t] = new_ptr
        ptr = new_ptr
    return decode.T                                       # (B, S)


def kernel(feats, mask, tags, transitions):
    feats = np.asarray(feats, dtype=np.float32)
    mask = np.asarray(mask)
    trans = np.asarray(transitions, dtype=np.float32)
    part = _forward_on_device(feats, trans)
    return _host_backtrack(part, feats, mask, trans)


# revision 8
# speedup vs baseline: 1.0712x; 1.0712x over previous
"""Viterbi CRF decode kernel for Trainium2, data-parallel over batch on 8 cores.

Device computes the forward Viterbi max-plus scan, storing the full fp32
partition history; the host recomputes backpointers lazily along the decoded
path (bit-exact vs the jax reference). See _host_backtrack.

Per-core structure (16 batches, partitions p=(b2,tag), pairs c in 0..7):
  cur(t) lives in PSUM, built as  base(f+trans) + replicated part(t-1):
    - base chunks prebuilt on GPSIMD (bulk), injected into PSUM by a PE
      identity matmul (start=True), prefetched `depth` steps ahead,
      off the critical chain
    - part(t-1) replication: diag(part) = part-bcast * diag-mask (DVE,
      last pair of each group on ACT via scale-AP multiply), then one PE
      matmul with block-ones lhsT accumulates it onto the base
      (start=False): PSUM = (f + trans) + part, the reference's add order;
      verified bit-exact on hardware
    - DVE tensor_reduce(max) over i -> part(t) -> part_hist
  Three independent chain groups (pairs 3/3/2) pipeline across DVE/PE/ACT
  so each group's diag->matmul->reduce round trip overlaps the others.
"""

import os
import sys

sys.path.insert(0, "/opt/trn_rl_repo")

import numpy as np

import concourse.bass as bass
import concourse.mybir as mybir
import concourse.tile as tile
from concourse.vector_clock import ScopedClock

B, S, T = 128, 512, 64
START_TAG, STOP_TAG = T - 2, T - 1
N_CORES = 8
BPC = B // N_CORES          # batches per core = 16
NCH = BPC // 2              # chain pairs per core = 8

_F32 = mybir.dt.float32

# tuned schedule parameters (sim-swept)
CFG = dict(
    groups=[(0, 3), (3, 3), (6, 2)],
    G=3,
    binj="pe",
    diag="dve",
    tchunk=4,
    depth=2,
    dsplit=1,
)


def _patch_tile_drain():
    """walrus in this toolchain rejects >1-2 sem waits on one CTRL
    instruction; split the TileContext tail-drain waits one-per-nop."""

    def _patched(self, tick_clock, wait_clock):
        carrier = self.nc.sync.nop()
        wait_clock.add_sem_waits(
            carrier.ins, ScopedClock({None: tick_clock.global_clock})
        )
        si = carrier.ins.sync_info
        waits = list(si.on_wait) if si and si.on_wait else []
        upds = list(si.on_update) if si and si.on_update else []
        if len(waits) > 1:
            carrier.ins.sync_info = mybir.SyncInfo(on_wait=[waits[0]], on_update=upds)
            for w in waits[1:]:
                n = self.nc.sync.nop()
                n.ins.sync_info = mybir.SyncInfo(on_wait=[w], on_update=[])
        self.nc.sync.drain()
        self.nc.all_engine_barrier()
        assert self.sems is not None
        popped = self.nc._tile_sem_poison_stack.pop()
        assert popped is self._sem_poison
        self.nc.all_engine_barrier()

    tile.TileContext._drain_and_barrier = _patched

    orig_add = tile.TileContext._add_instruction

    def _add_split(self, inst):
        si = getattr(inst, "sync_info", None)
        waits = list(si.on_wait) if si and si.on_wait else []
        lim = 1
        if len(waits) > lim:
            head, rest = waits[:lim], waits[lim:]
            for w in rest:
                carrier = mybir.InstNoOp(
                    name=self.nc.get_next_instruction_name(),
                    sync_info=mybir.SyncInfo(on_wait=[w], on_update=[]),
                    bass_nofuse=True,
                    engine=inst.engine,
                )
                orig_add(self, carrier)
            inst.sync_info = mybir.SyncInfo(
                on_wait=head, on_update=list(si.on_update or [])
            )
        orig_add(self, inst)

    tile.TileContext._add_instruction = _add_split


_patch_tile_drain()


def build_forward_kernel(cfg=None):
    """One NeuronCore's forward-scan bass module (config-C structure)."""
    cfg = dict(CFG, **(cfg or {}))
    if "groups" in cfg and cfg["groups"]:
        groups = cfg["groups"]            # list of (c0, npairs)
    else:
        G0 = cfg["G"]
        PPG0 = NCH // G0
        groups = [(g * PPG0, PPG0) for g in range(G0)]
    G = len(groups)
    binj = cfg["binj"]
    diag_eng = cfg["diag"]
    if isinstance(diag_eng, str):
        diag_eng = [diag_eng] * G
    TCHUNK = cfg["tchunk"]
    D = cfg["depth"]

    nc = bass.Bass()
    featsT = nc.declare_dram_parameter("featsT", [128, S, NCH], _F32, isOutput=False)
    transT = nc.declare_dram_parameter("transT", [T, T], _F32, isOutput=False)
    ident = nc.declare_dram_parameter("ident", [128, 128], _F32, isOutput=False)
    bones = nc.declare_dram_parameter("bones", [128, 128], _F32, isOutput=False)
    dmask = nc.declare_dram_parameter("dmask", [128, T], _F32, isOutput=False)
    parts = nc.declare_dram_parameter("parts", [128, S * NCH], _F32, isOutput=True)

    nchunks = (S + TCHUNK - 1) // TCHUNK

    with tile.TileContext(nc) as tc:
        with (
            tc.tile_pool(name="const", bufs=1) as constp,
            tc.tile_pool(name="hist", bufs=1) as histp,
            tc.tile_pool(name="ft", bufs=1) as ftp,
            tc.tile_pool(name="base", bufs=2) as basep,
            tc.tile_pool(name="work", bufs=1) as workp,
            tc.tile_pool(name="psum", bufs=1, space="PSUM") as psump,
        ):
            transRep = constp.tile([128, T], _F32, tag="transRep")
            nc.sync.dma_start(transRep[0:64, :], transT[:, :])
            nc.sync.dma_start(transRep[64:128, :], transT[:, :])
            identity = constp.tile([128, 128], _F32, tag="identity")
            nc.sync.dma_start(identity[:], ident[:])
            bones_sb = constp.tile([128, 128], _F32, tag="bones")
            nc.sync.dma_start(bones_sb[:], bones[:])
            dmask_sb = constp.tile([128, T], _F32, tag="dmask")
            nc.sync.dma_start(dmask_sb[:], dmask[:])

            part_hist = histp.tile([128, S * NCH], _F32, tag="part_hist")
            ft_all = ftp.tile([128, S, NCH], _F32, tag="ft")
            nc.sync.dma_start(ft_all[:], featsT[:])

            base_tiles = []

            def build_base(n):
                t0 = n * TCHUNK
                tn = min(TCHUNK, S - t0)
                ft = ft_all[:, t0:t0 + tn, :].rearrange("p s c -> p (s c)")
                bt = basep.tile([128, TCHUNK * NCH, T], _F32, tag="base")
                in0 = ft.unsqueeze(2).broadcast_to([128, tn * NCH, T])
                in1 = transRep[:].unsqueeze(1).broadcast_to([128, tn * NCH, T])
                nc.gpsimd.tensor_tensor(
                    bt[:, 0:tn * NCH, :], in0, in1, mybir.AluOpType.add
                )
                return bt

            base_tiles.append(build_base(0))
            if nchunks > 1:
                base_tiles.append(build_base(1))

            scr2 = [
                workp.tile([128, npair, T], _F32, name=f"scr2_{g}",
                           tag=f"scr2_{g}")
                for g, (c0g, npair) in enumerate(groups)
            ]
            cur = [
                [
                    psump.tile([128, npair, T], _F32, name=f"cur{g}_{d}",
                               tag=f"cur{g}_{d}")
                    for d in range(D)
                ]
                for g, (c0g, npair) in enumerate(groups)
            ]

            def inject_base(g, t):
                c0, npair = groups[g]
                bt = base_tiles[t // TCHUNK]
                trel = t % TCHUNK
                src = bt[:, trel * NCH + c0:trel * NCH + c0 + npair, :]
                dst = cur[g][t % D]
                if binj == "act":
                    nc.scalar.copy(dst[:], src)
                else:  # 'pe'
                    nc.tensor.matmul(
                        dst[:].rearrange("p c i -> p (c i)"),
                        identity[:],
                        src.rearrange("p c i -> p (c i)"),
                        start=True,
                        stop=False,
                        skip_group_check=True,
                    )

            DSPLIT = cfg.get("dsplit", 0)

            def accum_part(g, t):
                c0, npair = groups[g]
                col = (t - 1) * NCH + c0
                np_view = part_hist[:, col:col + npair]
                nd = npair - DSPLIT
                in0 = np_view[:, 0:nd].unsqueeze(2).broadcast_to([128, nd, T])
                in1 = dmask_sb[:].unsqueeze(1).broadcast_to([128, nd, T])
                eng = nc.vector if diag_eng[g] == "dve" else nc.gpsimd
                eng.tensor_tensor(scr2[g][:, 0:nd, :], in0, in1,
                                  mybir.AluOpType.mult)
                for cc in range(nd, npair):
                    nc.scalar.mul(
                        scr2[g][:, cc, :], dmask_sb[:], np_view[:, cc:cc + 1]
                    )
                nc.tensor.matmul(
                    cur[g][t % D][:].rearrange("p c i -> p (c i)"),
                    bones_sb[:],
                    scr2[g][:].rearrange("p c i -> p (c i)"),
                    start=False,
                    stop=True,
                    skip_group_check=True,
                )

            nc.vector.tensor_scalar_add(
                part_hist[:, 0:NCH],
                ft_all[:, 0, :],
                transRep[:, START_TAG:START_TAG + 1],
            )

            for t in range(1, min(D + 1, S)):
                for g in range(G):
                    inject_base(g, t)

            def emit_tr(g, t):
                c0, npair = groups[g]
                nc.vector.tensor_reduce(
                    part_hist[:, t * NCH + c0:t * NCH + c0 + npair],
                    cur[g][t % D][:],
                    axis=mybir.AxisListType.X,
                    op=mybir.AluOpType.max,
                )

            skew = cfg.get("skew", False) and G == 2
            if not skew:
                for t in range(1, S):
                    if t % TCHUNK == 0:
                        n = t // TCHUNK
                        if n + 1 < nchunks and n >= 1:
                            base_tiles.append(build_base(n + 1))
                    for g in range(G):
                        accum_part(g, t)
                        emit_tr(g, t)
                        if t + D < S:
                            inject_base(g, t + D)
            else:
                # half-step phase skew: group 1's TR trails by one emission
                # slot so TRs fill the diag->matmul round-trip bubbles.
                accum_part(0, 1)
                accum_part(1, 1)
                emit_tr(0, 1)
                if 1 + D < S:
                    inject_base(0, 1 + D)
                for t in range(2, S):
                    if t % TCHUNK == 0:
                        n = t // TCHUNK
                        if n + 1 < nchunks and n >= 1:
                            base_tiles.append(build_base(n + 1))
                    accum_part(0, t)
                    emit_tr(1, t - 1)
                    if t - 1 + D < S:
                        inject_base(1, t - 1 + D)
                    accum_part(1, t)
                    emit_tr(0, t)
                    if t + D < S:
                        inject_base(0, t + D)
                emit_tr(1, S - 1)

            nc.sync.dma_start(parts[:], part_hist[:])

    return nc


_FWD_CACHE = {}
LAST_EXEC_NS = None


def _sim_exec_ns(nc):
    """Cost-model (TimelineSim) predicted per-core duration in ns."""
    try:
        from concourse.timeline_sim import TimelineSim

        return int(TimelineSim(nc, no_exec=True).simulate())
    except Exception:
        return None


def _forward_on_device(feats_np, trans_np):
    """Run the forward scan on 8 cores. Returns part_hist [S, B, T] f32."""
    global LAST_EXEC_NS
    from concourse.bass_utils import run_bass_kernel_spmd

    if "nc" not in _FWD_CACHE:
        _FWD_CACHE["nc"] = build_forward_kernel()
    nc = _FWD_CACHE["nc"]

    if "sim_ns" not in _FWD_CACHE:
        _FWD_CACHE["sim_ns"] = _sim_exec_ns(nc)

    transT = np.ascontiguousarray(trans_np.T)
    ident = np.eye(128, dtype=np.float32)
    bones = np.zeros((128, 128), dtype=np.float32)
    bones[0:64, 0:64] = 1.0
    bones[64:128, 64:128] = 1.0
    dmask = np.zeros((128, T), dtype=np.float32)
    for u in range(T):
        dmask[u, u] = 1.0
        dmask[64 + u, u] = 1.0

    in_maps = []
    for k in range(N_CORES):
        shard = feats_np[k * BPC:(k + 1) * BPC]          # (16, S, T)
        # featsT[b2*64 + j, s, c] = shard[b2*8 + c, s, j]
        ft = np.ascontiguousarray(
            shard.reshape(2, NCH, S, T).transpose(0, 3, 2, 1).reshape(128, S, NCH)
        )
        in_maps.append(
            {"featsT": ft, "transT": transT, "ident": ident,
             "bones": bones, "dmask": dmask}
        )

    trace = bool(os.environ.get("CRF_TRACE"))
    res = run_bass_kernel_spmd(nc, in_maps, list(range(N_CORES)), trace=trace)
    if res.exec_time_ns is not None:
        LAST_EXEC_NS = res.exec_time_ns
    elif _FWD_CACHE["sim_ns"] is not None:
        LAST_EXEC_NS = _FWD_CACHE["sim_ns"]

    part = np.empty((S, B, T), dtype=np.float32)
    for k in range(N_CORES):
        p = res.results[k]["parts"].reshape(128, S, NCH)  # [(b2,j), t, c]
        p = p.reshape(2, T, S, NCH)                       # [b2, j, t, c]
        part[:, k * BPC:(k + 1) * BPC, :] = (
            p.transpose(2, 0, 3, 1).reshape(S, BPC, T)
        )
    return part


def _host_backtrack(part, feats, mask, trans):
    """Backpointer recompute + backtrack, bit-exact vs the jax reference."""
    f32 = np.float32
    lengths = mask.astype(np.int64).sum(axis=1)          # (B,)
    last_pos = lengths - 1
    bidx = np.arange(B)

    last_partition = part[last_pos, bidx, :]             # (B, T)
    last_vals = last_partition + trans[:, STOP_TAG][None, :].astype(f32)
    pointer0 = np.argmax(last_vals, axis=1).astype(np.int32)

    decode = np.zeros((S, B), dtype=np.int32)
    decode[S - 1] = pointer0
    ptr = pointer0
    trans_T = np.ascontiguousarray(trans.T)              # trans_T[j, i] = trans[i, j]
    for t in range(S - 2, -1, -1):
        jstar = ptr                                       # decode[t+1]
        fcol = feats[bidx, t + 1, jstar].astype(f32)      # (B,)
        cur = (fcol[:, None] + trans_T[jstar]) + part[t]  # (B, T) f32
        bp_val = np.argmax(cur, axis=1).astype(np.int32)
        new_ptr = np.where(
            t == last_pos, pointer0,
            np.where(t >= lengths, 0, bp_val)
        ).astype(np.int32)
        decode[t] = new_ptr
        ptr = new_ptr
    return decode.T                                       # (B, S)


def kernel(feats, mask, tags, transitions):
    feats = np.asarray(feats, dtype=np.float32)
    mask = np.asarray(mask)
    trans = np.asarray(transitions, dtype=np.float32)
    part = _forward_on_device(feats, trans)
    return _host_backtrack(part, feats, mask, trans)


# revision 10
# speedup vs baseline: 1.2580x; 1.1743x over previous
"""Viterbi CRF decode kernel for Trainium2, data-parallel over batch on 8 cores.

Device computes the forward Viterbi max-plus scan, storing the full fp32
partition history; the host recomputes backpointers lazily along the decoded
path (bit-exact vs the jax reference). See _host_backtrack.

Per-core structure (16 batches, partitions p=(b2,tag), pairs c in 0..7):
  cur(t) lives in PSUM, built as  base(f+trans) + replicated part(t-1):
    - base chunks prebuilt on GPSIMD (bulk), injected into PSUM by a PE
      identity matmul (start=True), prefetched `depth` steps ahead,
      off the critical chain
    - part(t-1) replication: diag(part) = part-bcast * diag-mask (DVE,
      last pair of each group on ACT via scale-AP multiply), then one PE
      matmul with block-ones lhsT accumulates it onto the base
      (start=False): PSUM = (f + trans) + part, the reference's add order;
      verified bit-exact on hardware
    - DVE tensor_reduce(max) over i -> part(t) -> part_hist
  Three independent chain groups (pairs 3/3/2) pipeline across DVE/PE/ACT
  so each group's diag->matmul->reduce round trip overlaps the others.
"""

import os
import sys

sys.path.insert(0, "/opt/trn_rl_repo")

import numpy as np

import concourse.bass as bass
import concourse.mybir as mybir
import concourse.tile as tile
from concourse.vector_clock import ScopedClock

B, S, T = 128, 512, 64
START_TAG, STOP_TAG = T - 2, T - 1
N_CORES = 8
BPC = B // N_CORES          # batches per core = 16
NCH = BPC // 2              # chain pairs per core = 8

_F32 = mybir.dt.float32

# tuned schedule parameters (sim-swept)
CFG = dict(
    groups=[(0, 2), (2, 2), (4, 2), (6, 2)],
    G=4,
    binj="pe",
    diag="dve",
    tchunk=4,
    depth=2,
    dsplit=1,
)


def _patch_tile_drain():
    """walrus in this toolchain rejects >1-2 sem waits on one CTRL
    instruction; split the TileContext tail-drain waits one-per-nop."""

    def _patched(self, tick_clock, wait_clock):
        carrier = self.nc.sync.nop()
        wait_clock.add_sem_waits(
            carrier.ins, ScopedClock({None: tick_clock.global_clock})
        )
        si = carrier.ins.sync_info
        waits = list(si.on_wait) if si and si.on_wait else []
        upds = list(si.on_update) if si and si.on_update else []
        if len(waits) > 1:
            carrier.ins.sync_info = mybir.SyncInfo(on_wait=[waits[0]], on_update=upds)
            for w in waits[1:]:
                n = self.nc.sync.nop()
                n.ins.sync_info = mybir.SyncInfo(on_wait=[w], on_update=[])
        self.nc.sync.drain()
        self.nc.all_engine_barrier()
        assert self.sems is not None
        popped = self.nc._tile_sem_poison_stack.pop()
        assert popped is self._sem_poison
        self.nc.all_engine_barrier()

    tile.TileContext._drain_and_barrier = _patched

    orig_add = tile.TileContext._add_instruction

    def _add_split(self, inst):
        si = getattr(inst, "sync_info", None)
        waits = list(si.on_wait) if si and si.on_wait else []
        lim = 1
        if len(waits) > lim:
            head, rest = waits[:lim], waits[lim:]
            for w in rest:
                carrier = mybir.InstNoOp(
                    name=self.nc.get_next_instruction_name(),
                    sync_info=mybir.SyncInfo(on_wait=[w], on_update=[]),
                    bass_nofuse=True,
                    engine=inst.engine,
                )
                orig_add(self, carrier)
            inst.sync_info = mybir.SyncInfo(
                on_wait=head, on_update=list(si.on_update or [])
            )
        orig_add(self, inst)

    tile.TileContext._add_instruction = _add_split


_patch_tile_drain()


def build_forward_kernel(cfg=None, glens=None):
    """One NeuronCore's forward-scan bass module (config-C structure).

    glens[g] = number of scan steps group g actually needs (max sequence
    length over its batches). Steps beyond glens[g] are never read by the
    host backtrack, so the group simply retires early.
    """
    cfg = dict(CFG, **(cfg or {}))
    if "groups" in cfg and cfg["groups"]:
        groups = cfg["groups"]            # list of (c0, npairs)
    else:
        G0 = cfg["G"]
        PPG0 = NCH // G0
        groups = [(g * PPG0, PPG0) for g in range(G0)]
    G = len(groups)
    binj = cfg["binj"]
    diag_eng = cfg["diag"]
    if isinstance(diag_eng, str):
        diag_eng = [diag_eng] * G
    TCHUNK = cfg["tchunk"]
    D = cfg["depth"]
    if glens is None:
        glens = [S] * G
    assert len(glens) == G
    glens = [min(S, max(int(x), D + 2)) for x in glens]
    LMAX = max(glens)

    nc = bass.Bass()
    featsT = nc.declare_dram_parameter("featsT", [128, S, NCH], _F32, isOutput=False)
    transT = nc.declare_dram_parameter("transT", [T, T], _F32, isOutput=False)
    ident = nc.declare_dram_parameter("ident", [128, 128], _F32, isOutput=False)
    bones = nc.declare_dram_parameter("bones", [128, 128], _F32, isOutput=False)
    dmask = nc.declare_dram_parameter("dmask", [128, T], _F32, isOutput=False)
    parts = nc.declare_dram_parameter("parts", [128, S * NCH], _F32, isOutput=True)

    nchunks = (LMAX + TCHUNK - 1) // TCHUNK

    with tile.TileContext(nc) as tc:
        with (
            tc.tile_pool(name="const", bufs=1) as constp,
            tc.tile_pool(name="hist", bufs=1) as histp,
            tc.tile_pool(name="ft", bufs=1) as ftp,
            tc.tile_pool(name="base", bufs=2) as basep,
            tc.tile_pool(name="work", bufs=1) as workp,
            tc.tile_pool(name="psum", bufs=1, space="PSUM") as psump,
        ):
            transRep = constp.tile([128, T], _F32, tag="transRep")
            nc.sync.dma_start(transRep[0:64, :], transT[:, :])
            nc.sync.dma_start(transRep[64:128, :], transT[:, :])
            identity = constp.tile([128, 128], _F32, tag="identity")
            nc.sync.dma_start(identity[:], ident[:])
            bones_sb = constp.tile([128, 128], _F32, tag="bones")
            nc.sync.dma_start(bones_sb[:], bones[:])
            dmask_sb = constp.tile([128, T], _F32, tag="dmask")
            nc.sync.dma_start(dmask_sb[:], dmask[:])

            part_hist = histp.tile([128, S * NCH], _F32, tag="part_hist")
            ft_all = ftp.tile([128, S, NCH], _F32, tag="ft")
            nc.sync.dma_start(ft_all[:], featsT[:])

            base_tiles = []

            def build_base(n):
                t0 = n * TCHUNK
                tn = min(TCHUNK, S - t0)
                ft = ft_all[:, t0:t0 + tn, :].rearrange("p s c -> p (s c)")
                bt = basep.tile([128, TCHUNK * NCH, T], _F32, tag="base")
                in0 = ft.unsqueeze(2).broadcast_to([128, tn * NCH, T])
                in1 = transRep[:].unsqueeze(1).broadcast_to([128, tn * NCH, T])
                nc.gpsimd.tensor_tensor(
                    bt[:, 0:tn * NCH, :], in0, in1, mybir.AluOpType.add
                )
                return bt

            base_tiles.append(build_base(0))
            if nchunks > 1:
                base_tiles.append(build_base(1))

            scr2 = [
                workp.tile([128, npair, T], _F32, name=f"scr2_{g}",
                           tag=f"scr2_{g}")
                for g, (c0g, npair) in enumerate(groups)
            ]
            cur = [
                [
                    psump.tile([128, npair, T], _F32, name=f"cur{g}_{d}",
                               tag=f"cur{g}_{d}")
                    for d in range(D)
                ]
                for g, (c0g, npair) in enumerate(groups)
            ]

            def inject_base(g, t):
                c0, npair = groups[g]
                bt = base_tiles[t // TCHUNK]
                trel = t % TCHUNK
                src = bt[:, trel * NCH + c0:trel * NCH + c0 + npair, :]
                dst = cur[g][t % D]
                if binj == "act":
                    nc.scalar.copy(dst[:], src)
                else:  # 'pe'
                    nc.tensor.matmul(
                        dst[:].rearrange("p c i -> p (c i)"),
                        identity[:],
                        src.rearrange("p c i -> p (c i)"),
                        start=True,
                        stop=False,
                        skip_group_check=True,
                    )

            DSPLIT = cfg.get("dsplit", 0)

            def accum_part(g, t):
                c0, npair = groups[g]
                col = (t - 1) * NCH + c0
                np_view = part_hist[:, col:col + npair]
                nd = npair - DSPLIT
                in0 = np_view[:, 0:nd].unsqueeze(2).broadcast_to([128, nd, T])
                in1 = dmask_sb[:].unsqueeze(1).broadcast_to([128, nd, T])
                eng = nc.vector if diag_eng[g] == "dve" else nc.gpsimd
                eng.tensor_tensor(scr2[g][:, 0:nd, :], in0, in1,
                                  mybir.AluOpType.mult)
                for cc in range(nd, npair):
                    nc.scalar.mul(
                        scr2[g][:, cc, :], dmask_sb[:], np_view[:, cc:cc + 1]
                    )
                nc.tensor.matmul(
                    cur[g][t % D][:].rearrange("p c i -> p (c i)"),
                    bones_sb[:],
                    scr2[g][:].rearrange("p c i -> p (c i)"),
                    start=False,
                    stop=True,
                    skip_group_check=True,
                )

            nc.vector.tensor_scalar_add(
                part_hist[:, 0:NCH],
                ft_all[:, 0, :],
                transRep[:, START_TAG:START_TAG + 1],
            )

            for t in range(1, D + 1):
                for g in range(G):
                    if t < glens[g]:
                        inject_base(g, t)

            def emit_tr(g, t):
                c0, npair = groups[g]
                nc.vector.tensor_reduce(
                    part_hist[:, t * NCH + c0:t * NCH + c0 + npair],
                    cur[g][t % D][:],
                    axis=mybir.AxisListType.X,
                    op=mybir.AluOpType.max,
                )

            skew = cfg.get("skew", False) and G == 2
            if not skew:
                for t in range(1, LMAX):
                    if t % TCHUNK == 0:
                        n = t // TCHUNK
                        if n + 1 < nchunks and n >= 1:
                            base_tiles.append(build_base(n + 1))
                    for g in range(G):
                        if t >= glens[g]:
                            continue
                        accum_part(g, t)
                        emit_tr(g, t)
                        if t + D < glens[g]:
                            inject_base(g, t + D)
            else:
                # half-step phase skew: group 1's TR trails by one emission
                # slot so TRs fill the diag->matmul round-trip bubbles.
                accum_part(0, 1)
                accum_part(1, 1)
                emit_tr(0, 1)
                if 1 + D < S:
                    inject_base(0, 1 + D)
                for t in range(2, S):
                    if t % TCHUNK == 0:
                        n = t // TCHUNK
                        if n + 1 < nchunks and n >= 1:
                            base_tiles.append(build_base(n + 1))
                    accum_part(0, t)
                    emit_tr(1, t - 1)
                    if t - 1 + D < S:
                        inject_base(1, t - 1 + D)
                    accum_part(1, t)
                    emit_tr(0, t)
                    if t + D < S:
                        inject_base(0, t + D)
                emit_tr(1, S - 1)

            nc.sync.dma_start(parts[:], part_hist[:])

    return nc


_FWD_CACHE = {}
LAST_EXEC_NS = None


def _sim_exec_ns(nc):
    """Cost-model (TimelineSim) predicted per-core duration in ns."""
    try:
        from concourse.timeline_sim import TimelineSim

        return int(TimelineSim(nc, no_exec=True).simulate())
    except Exception:
        return None


def _plan_batches(lengths):
    """Sort batches by length (desc) into chain-group slots so short
    groups retire early. Returns (perm, glens): perm[new_idx] = orig batch,
    new_idx = core*16 + b2*8 + c; glens[g] = steps group g needs."""
    groups = CFG["groups"]
    order = np.argsort(-lengths, kind="stable")
    perm = np.empty(B, dtype=np.int64)
    glens = []
    pos = 0
    for g, (c0, npair) in enumerate(groups):
        n_slot = 2 * npair * N_CORES
        slot = order[pos:pos + n_slot]
        glens.append(int(lengths[slot].max()))
        for k in range(N_CORES):
            csl = slot[k * 2 * npair:(k + 1) * 2 * npair]
            for idx, borig in enumerate(csl):
                c = c0 + idx % npair
                b2 = idx // npair
                perm[k * BPC + b2 * NCH + c] = borig
        pos += n_slot
    return perm, glens


def _forward_on_device(feats_np, trans_np, glens):
    """Run the forward scan on 8 cores. Returns part_hist [S, B, T] f32
    (batch dim in the permuted order of feats_np)."""
    global LAST_EXEC_NS
    from concourse.bass_utils import run_bass_kernel_spmd

    key = tuple(glens)
    if key not in _FWD_CACHE:
        nc = build_forward_kernel(glens=glens)
        _FWD_CACHE[key] = (nc, _sim_exec_ns(nc))
    nc, sim_ns = _FWD_CACHE[key]

    transT = np.ascontiguousarray(trans_np.T)
    ident = np.eye(128, dtype=np.float32)
    bones = np.zeros((128, 128), dtype=np.float32)
    bones[0:64, 0:64] = 1.0
    bones[64:128, 64:128] = 1.0
    dmask = np.zeros((128, T), dtype=np.float32)
    for u in range(T):
        dmask[u, u] = 1.0
        dmask[64 + u, u] = 1.0

    in_maps = []
    for k in range(N_CORES):
        shard = feats_np[k * BPC:(k + 1) * BPC]          # (16, S, T)
        # featsT[b2*64 + j, s, c] = shard[b2*8 + c, s, j]
        ft = np.ascontiguousarray(
            shard.reshape(2, NCH, S, T).transpose(0, 3, 2, 1).reshape(128, S, NCH)
        )
        in_maps.append(
            {"featsT": ft, "transT": transT, "ident": ident,
             "bones": bones, "dmask": dmask}
        )

    trace = bool(os.environ.get("CRF_TRACE"))
    res = run_bass_kernel_spmd(nc, in_maps, list(range(N_CORES)), trace=trace)
    if res.exec_time_ns is not None:
        LAST_EXEC_NS = res.exec_time_ns
    elif sim_ns is not None:
        LAST_EXEC_NS = sim_ns

    part = np.empty((S, B, T), dtype=np.float32)
    for k in range(N_CORES):
        p = res.results[k]["parts"].reshape(128, S, NCH)  # [(b2,j), t, c]
        p = p.reshape(2, T, S, NCH)                       # [b2, j, t, c]
        part[:, k * BPC:(k + 1) * BPC, :] = (
            p.transpose(2, 0, 3, 1).reshape(S, BPC, T)
        )
    return part


def _host_backtrack(part, feats, mask, trans):
    """Backpointer recompute + backtrack, bit-exact vs the jax reference."""
    f32 = np.float32
    lengths = mask.astype(np.int64).sum(axis=1)          # (B,)
    last_pos = lengths - 1
    bidx = np.arange(B)

    last_partition = part[last_pos, bidx, :]             # (B, T)
    last_vals = last_partition + trans[:, STOP_TAG][None, :].astype(f32)
    pointer0 = np.argmax(last_vals, axis=1).astype(np.int32)

    decode = np.zeros((S, B), dtype=np.int32)
    decode[S - 1] = pointer0
    ptr = pointer0
    trans_T = np.ascontiguousarray(trans.T)              # trans_T[j, i] = trans[i, j]
    for t in range(S - 2, -1, -1):
        jstar = ptr                                       # decode[t+1]
        fcol = feats[bidx, t + 1, jstar].astype(f32)      # (B,)
        cur = (fcol[:, None] + trans_T[jstar]) + part[t]  # (B, T) f32
        bp_val = np.argmax(cur, axis=1).astype(np.int32)
        new_ptr = np.where(
            t == last_pos, pointer0,
            np.where(t >= lengths, 0, bp_val)
        ).astype(np.int32)
        decode[t] = new_ptr
        ptr = new_ptr
    return decode.T                                       # (B, S)


def kernel(feats, mask, tags, transitions):
    feats = np.asarray(feats, dtype=np.float32)
    mask = np.asarray(mask)
    trans = np.asarray(transitions, dtype=np.float32)
    lengths = mask.astype(np.int64).sum(axis=1)
    perm, glens = _plan_batches(lengths)
    feats_p = np.ascontiguousarray(feats[perm])
    mask_p = np.ascontiguousarray(mask[perm])
    part = _forward_on_device(feats_p, trans, glens)
    decode_p = _host_backtrack(part, feats_p, mask_p, trans)
    out = np.empty_like(decode_p)
    out[perm] = decode_p
    return out


# revision 13
# speedup vs baseline: 1.2947x; 1.0292x over previous
"""Viterbi CRF decode kernel for Trainium2, data-parallel over batch on 8 cores.

Device computes the forward Viterbi max-plus scan, storing the full fp32
partition history; the host recomputes backpointers lazily along the decoded
path (bit-exact vs the jax reference). See _host_backtrack.

Per-core structure (16 batches, partitions p=(b2,tag), pairs c in 0..7):
  cur(t) lives in PSUM, built as  base(f+trans) + replicated part(t-1):
    - base chunks prebuilt on GPSIMD (bulk), injected into PSUM by a PE
      identity matmul (start=True), prefetched `depth` steps ahead,
      off the critical chain
    - part(t-1) replication: diag(part) = part-bcast * diag-mask (DVE,
      last pair of each group on ACT via scale-AP multiply), then one PE
      matmul with block-ones lhsT accumulates it onto the base
      (start=False): PSUM = (f + trans) + part, the reference's add order;
      verified bit-exact on hardware
    - DVE tensor_reduce(max) over i -> part(t) -> part_hist
  Three independent chain groups (pairs 3/3/2) pipeline across DVE/PE/ACT
  so each group's diag->matmul->reduce round trip overlaps the others.
"""

import os
import sys

sys.path.insert(0, "/opt/trn_rl_repo")

import numpy as np

import concourse.bass as bass
import concourse.mybir as mybir
import concourse.tile as tile
from concourse.vector_clock import ScopedClock

B, S, T = 128, 512, 64
START_TAG, STOP_TAG = T - 2, T - 1
N_CORES = 8
BPC = B // N_CORES          # batches per core = 16
NCH = BPC // 2              # chain pairs per core = 8

_F32 = mybir.dt.float32

# tuned schedule parameters (sim-swept)
CFG = dict(
    groups=[(0, 2), (2, 2), (4, 2), (6, 2)],
    G=4,
    binj="pe",
    diag="dve",
    tchunk=4,
    depth=2,
    dsplit=1,
)


def _patch_tile_drain():
    """walrus in this toolchain rejects >1-2 sem waits on one CTRL
    instruction; split the TileContext tail-drain waits one-per-nop."""

    def _patched(self, tick_clock, wait_clock):
        carrier = self.nc.sync.nop()
        wait_clock.add_sem_waits(
            carrier.ins, ScopedClock({None: tick_clock.global_clock})
        )
        si = carrier.ins.sync_info
        waits = list(si.on_wait) if si and si.on_wait else []
        upds = list(si.on_update) if si and si.on_update else []
        if len(waits) > 1:
            carrier.ins.sync_info = mybir.SyncInfo(on_wait=[waits[0]], on_update=upds)
            for w in waits[1:]:
                n = self.nc.sync.nop()
                n.ins.sync_info = mybir.SyncInfo(on_wait=[w], on_update=[])
        self.nc.sync.drain()
        self.nc.all_engine_barrier()
        assert self.sems is not None
        popped = self.nc._tile_sem_poison_stack.pop()
        assert popped is self._sem_poison
        self.nc.all_engine_barrier()

    tile.TileContext._drain_and_barrier = _patched

    orig_add = tile.TileContext._add_instruction

    def _add_split(self, inst):
        si = getattr(inst, "sync_info", None)
        waits = list(si.on_wait) if si and si.on_wait else []
        lim = 1
        if len(waits) > lim:
            head, rest = waits[:lim], waits[lim:]
            for w in rest:
                carrier = mybir.InstNoOp(
                    name=self.nc.get_next_instruction_name(),
                    sync_info=mybir.SyncInfo(on_wait=[w], on_update=[]),
                    bass_nofuse=True,
                    engine=inst.engine,
                )
                orig_add(self, carrier)
            inst.sync_info = mybir.SyncInfo(
                on_wait=head, on_update=list(si.on_update or [])
            )
        orig_add(self, inst)

    tile.TileContext._add_instruction = _add_split


_patch_tile_drain()


def build_forward_kernel(cfg=None, glens=None):
    """One NeuronCore's forward-scan bass module (config-C structure).

    glens[g] = number of scan steps group g actually needs (max sequence
    length over its batches). Steps beyond glens[g] are never read by the
    host backtrack, so the group simply retires early.
    """
    cfg = dict(CFG, **(cfg or {}))
    if "groups" in cfg and cfg["groups"]:
        groups = cfg["groups"]            # list of (c0, npairs)
    else:
        G0 = cfg["G"]
        PPG0 = NCH // G0
        groups = [(g * PPG0, PPG0) for g in range(G0)]
    G = len(groups)
    binj = cfg["binj"]
    if isinstance(binj, str):
        binj = [binj] * 8
    diag_eng = cfg["diag"]
    if isinstance(diag_eng, str):
        diag_eng = [diag_eng] * G
    TCHUNK = cfg["tchunk"]
    D = cfg["depth"]
    if glens is None:
        glens = [S] * G
    assert len(glens) == G
    def _norm(x):
        if isinstance(x, (list, tuple)):
            ls = [min(S, max(int(v), D + 2)) for v in x]
        else:
            ls = [min(S, max(int(x), D + 2))] * 2
        assert len(ls) == 2 and ls[0] >= ls[1]
        return ls
    glens = [_norm(x) for x in glens]
    LMAX = max(l[0] for l in glens)

    def nact(g, t):
        """active pairs of group g at step t (pair 0 outlives pair 1)."""
        return (1 if t < glens[g][0] else 0) + (1 if t < glens[g][1] else 0)

    nc = bass.Bass()
    featsT = nc.declare_dram_parameter("featsT", [128, S, NCH], _F32, isOutput=False)
    transT = nc.declare_dram_parameter("transT", [T, T], _F32, isOutput=False)
    ident = nc.declare_dram_parameter("ident", [128, 128], _F32, isOutput=False)
    bones = nc.declare_dram_parameter("bones", [128, 128], _F32, isOutput=False)
    dmask = nc.declare_dram_parameter("dmask", [128, T], _F32, isOutput=False)
    parts = nc.declare_dram_parameter("parts", [128, S * NCH], _F32, isOutput=True)

    nchunks = (LMAX + TCHUNK - 1) // TCHUNK

    with tile.TileContext(nc) as tc:
        with (
            tc.tile_pool(name="const", bufs=1) as constp,
            tc.tile_pool(name="hist", bufs=1) as histp,
            tc.tile_pool(name="ft", bufs=1) as ftp,
            tc.tile_pool(name="base", bufs=2) as basep,
            tc.tile_pool(name="work", bufs=1) as workp,
            tc.tile_pool(name="psum", bufs=1, space="PSUM") as psump,
        ):
            transRep = constp.tile([128, T], _F32, tag="transRep")
            nc.sync.dma_start(transRep[0:64, :], transT[:, :])
            nc.sync.dma_start(transRep[64:128, :], transT[:, :])
            identity = constp.tile([128, 128], _F32, tag="identity")
            nc.sync.dma_start(identity[:], ident[:])
            bones_sb = constp.tile([128, 128], _F32, tag="bones")
            nc.sync.dma_start(bones_sb[:], bones[:])
            dmask_sb = constp.tile([128, T], _F32, tag="dmask")
            nc.sync.dma_start(dmask_sb[:], dmask[:])

            part_hist = histp.tile([128, S * NCH], _F32, tag="part_hist")
            ft_all = ftp.tile([128, S, NCH], _F32, tag="ft")
            nc.sync.dma_start(ft_all[:], featsT[:])

            base_tiles = []

            def build_base(n):
                t0 = n * TCHUNK
                tn = min(TCHUNK, S - t0)
                ft = ft_all[:, t0:t0 + tn, :].rearrange("p s c -> p (s c)")
                bt = basep.tile([128, TCHUNK * NCH, T], _F32, tag="base")
                in0 = ft.unsqueeze(2).broadcast_to([128, tn * NCH, T])
                in1 = transRep[:].unsqueeze(1).broadcast_to([128, tn * NCH, T])
                nc.gpsimd.tensor_tensor(
                    bt[:, 0:tn * NCH, :], in0, in1, mybir.AluOpType.add
                )
                return bt

            base_tiles.append(build_base(0))
            if nchunks > 1:
                base_tiles.append(build_base(1))

            scr2 = [
                workp.tile([128, npair, T], _F32, name=f"scr2_{g}",
                           tag=f"scr2_{g}")
                for g, (c0g, npair) in enumerate(groups)
            ]
            share_tr = cfg.get("share_tr", False) and G % 2 == 0
            if share_tr:
                # groups 2t and 2t+1 share one PSUM tile; group 2t (longer
                # glen) occupies the leading columns so retirement keeps a
                # contiguous active prefix for the fused tensor_reduce.
                tilew = [groups[2 * tg][1] + groups[2 * tg + 1][1]
                         for tg in range(G // 2)]
                curT = [
                    [
                        psump.tile([128, tilew[tg], T], _F32,
                                   name=f"curT{tg}_{d}", tag=f"curT{tg}_{d}")
                        for d in range(D)
                    ]
                    for tg in range(G // 2)
                ]

                def cur_slice(g, t):
                    tg, half = divmod(g, 2)
                    off = 0 if half == 0 else groups[2 * tg][1]
                    npair = groups[g][1]
                    return curT[tg][t % D][:, off:off + npair, :]
            else:
                cur = [
                    [
                        psump.tile([128, npair, T], _F32, name=f"cur{g}_{d}",
                                   tag=f"cur{g}_{d}")
                        for d in range(D)
                    ]
                    for g, (c0g, npair) in enumerate(groups)
                ]

                def cur_slice(g, t):
                    return cur[g][t % D][:]

            def inject_base(g, t):
                c0, npair = groups[g]
                npair = min(npair, nact(g, t))
                bt = base_tiles[t // TCHUNK]
                trel = t % TCHUNK
                src = bt[:, trel * NCH + c0:trel * NCH + c0 + npair, :]
                dst = cur_slice(g, t)[:, 0:npair, :]
                bj = binj[g]
                if bj == "act":
                    nc.scalar.copy(dst, src)
                elif bj == "pool":
                    nc.gpsimd.tensor_copy(dst, src)
                else:  # 'pe'
                    nc.tensor.matmul(
                        dst.rearrange("p c i -> p (c i)"),
                        identity[:],
                        src.rearrange("p c i -> p (c i)"),
                        start=True,
                        stop=False,
                        skip_group_check=True,
                    )

            DSPLIT = cfg.get("dsplit", 0)

            def accum_part(g, t):
                c0, npair = groups[g]
                npair = min(npair, nact(g, t))
                col = (t - 1) * NCH + c0
                np_view = part_hist[:, col:col + npair]
                nd = npair - (DSPLIT if npair >= 2 else 0)
                in0 = np_view[:, 0:nd].unsqueeze(2).broadcast_to([128, nd, T])
                in1 = dmask_sb[:].unsqueeze(1).broadcast_to([128, nd, T])
                eng = nc.vector if diag_eng[g] == "dve" else nc.gpsimd
                eng.tensor_tensor(scr2[g][:, 0:nd, :], in0, in1,
                                  mybir.AluOpType.mult)
                for cc in range(nd, npair):
                    nc.scalar.mul(
                        scr2[g][:, cc, :], dmask_sb[:], np_view[:, cc:cc + 1]
                    )
                nc.tensor.matmul(
                    cur_slice(g, t)[:, 0:npair, :].rearrange("p c i -> p (c i)"),
                    bones_sb[:],
                    scr2[g][:, 0:npair, :].rearrange("p c i -> p (c i)"),
                    start=False,
                    stop=True,
                    skip_group_check=True,
                )

            nc.vector.tensor_scalar_add(
                part_hist[:, 0:NCH],
                ft_all[:, 0, :],
                transRep[:, START_TAG:START_TAG + 1],
            )

            for t in range(1, D + 1):
                for g in range(G):
                    if t < glens[g][0]:
                        inject_base(g, t)

            def emit_tr(g, t):
                c0, npair = groups[g]
                npair = min(npair, nact(g, t))
                nc.vector.tensor_reduce(
                    part_hist[:, t * NCH + c0:t * NCH + c0 + npair],
                    cur_slice(g, t)[:, 0:npair, :],
                    axis=mybir.AxisListType.X,
                    op=mybir.AluOpType.max,
                )

            def emit_tr_fused(tg, t):
                """One reduce over the active prefix of supertile tg.
                Valid because group 2tg's pairs precede group 2tg+1's in both
                the tile and part_hist (c0 ordering), and 2tg outlives 2tg+1."""
                ga, gb = 2 * tg, 2 * tg + 1
                c0a, npa = groups[ga]
                w = npa + (groups[gb][1] if t < glens[gb][0] else 0)
                nc.vector.tensor_reduce(
                    part_hist[:, t * NCH + c0a:t * NCH + c0a + w],
                    curT[tg][t % D][:, 0:w, :],
                    axis=mybir.AxisListType.X,
                    op=mybir.AluOpType.max,
                )

            skew = cfg.get("skew", False) and G == 2
            if not skew:
                for t in range(1, LMAX):
                    if t % TCHUNK == 0:
                        n = t // TCHUNK
                        if n + 1 < nchunks and n >= 1:
                            base_tiles.append(build_base(n + 1))
                    if share_tr:
                        for tg in range(G // 2):
                            ga, gb = 2 * tg, 2 * tg + 1
                            if t >= glens[ga][0]:
                                continue
                            accum_part(ga, t)
                            if t < glens[gb][0]:
                                accum_part(gb, t)
                            emit_tr_fused(tg, t)
                            if t + D < glens[ga][0]:
                                inject_base(ga, t + D)
                            if t + D < glens[gb][0]:
                                inject_base(gb, t + D)
                    else:
                        for g in range(G):
                            if t >= glens[g][0]:
                                continue
                            accum_part(g, t)
                            emit_tr(g, t)
                            if t + D < glens[g][0]:
                                inject_base(g, t + D)
            else:
                # half-step phase skew: group 1's TR trails by one emission
                # slot so TRs fill the diag->matmul round-trip bubbles.
                accum_part(0, 1)
                accum_part(1, 1)
                emit_tr(0, 1)
                if 1 + D < S:
                    inject_base(0, 1 + D)
                for t in range(2, S):
                    if t % TCHUNK == 0:
                        n = t // TCHUNK
                        if n + 1 < nchunks and n >= 1:
                            base_tiles.append(build_base(n + 1))
                    accum_part(0, t)
                    emit_tr(1, t - 1)
                    if t - 1 + D < S:
                        inject_base(1, t - 1 + D)
                    accum_part(1, t)
                    emit_tr(0, t)
                    if t + D < S:
                        inject_base(0, t + D)
                emit_tr(1, S - 1)

            nc.sync.dma_start(parts[:], part_hist[:])

    return nc


_FWD_CACHE = {}
LAST_EXEC_NS = None


def _sim_exec_ns(nc):
    """Cost-model (TimelineSim) predicted per-core duration in ns."""
    try:
        from concourse.timeline_sim import TimelineSim

        return int(TimelineSim(nc, no_exec=True).simulate())
    except Exception:
        return None


def _plan_batches(lengths):
    """Sort batches by length (desc) into chain-group slots so short
    groups retire early. Returns (perm, glens): perm[new_idx] = orig batch,
    new_idx = core*16 + b2*8 + c; glens[g] = steps group g needs."""
    groups = CFG["groups"]
    order = np.argsort(-lengths, kind="stable")
    perm = np.empty(B, dtype=np.int64)
    glens = []
    pos = 0
    for g, (c0, npair) in enumerate(groups):
        # pair q of this group takes the q-th 16-batch run (desc length),
        # so pair 0 outlives pair 1 and columns retire back-to-front.
        plens = []
        for q in range(npair):
            sub = order[pos:pos + 2 * N_CORES]
            plens.append(int(lengths[sub].max()))
            for k in range(N_CORES):
                perm[k * BPC + 0 * NCH + c0 + q] = sub[2 * k]
                perm[k * BPC + 1 * NCH + c0 + q] = sub[2 * k + 1]
            pos += 2 * N_CORES
        glens.append(plens)
    return perm, glens


def _forward_on_device(feats_np, trans_np, glens):
    """Run the forward scan on 8 cores. Returns part_hist [S, B, T] f32
    (batch dim in the permuted order of feats_np)."""
    global LAST_EXEC_NS
    from concourse.bass_utils import run_bass_kernel_spmd

    key = tuple(tuple(x) for x in glens)
    if key not in _FWD_CACHE:
        nc = build_forward_kernel(glens=glens)
        _FWD_CACHE[key] = (nc, _sim_exec_ns(nc))
    nc, sim_ns = _FWD_CACHE[key]

    transT = np.ascontiguousarray(trans_np.T)
    ident = np.eye(128, dtype=np.float32)
    bones = np.zeros((128, 128), dtype=np.float32)
    bones[0:64, 0:64] = 1.0
    bones[64:128, 64:128] = 1.0
    dmask = np.zeros((128, T), dtype=np.float32)
    for u in range(T):
        dmask[u, u] = 1.0
        dmask[64 + u, u] = 1.0

    in_maps = []
    for k in range(N_CORES):
        shard = feats_np[k * BPC:(k + 1) * BPC]          # (16, S, T)
        # featsT[b2*64 + j, s, c] = shard[b2*8 + c, s, j]
        ft = np.ascontiguousarray(
            shard.reshape(2, NCH, S, T).transpose(0, 3, 2, 1).reshape(128, S, NCH)
        )
        in_maps.append(
            {"featsT": ft, "transT": transT, "ident": ident,
             "bones": bones, "dmask": dmask}
        )

    trace = bool(os.environ.get("CRF_TRACE"))
    res = run_bass_kernel_spmd(nc, in_maps, list(range(N_CORES)), trace=trace)
    if res.exec_time_ns is not None:
        LAST_EXEC_NS = res.exec_time_ns
    elif sim_ns is not None:
        LAST_EXEC_NS = sim_ns

    part = np.empty((S, B, T), dtype=np.float32)
    for k in range(N_CORES):
        p = res.results[k]["parts"].reshape(128, S, NCH)  # [(b2,j), t, c]
        p = p.reshape(2, T, S, NCH)                       # [b2, j, t, c]
        part[:, k * BPC:(k + 1) * BPC, :] = (
            p.transpose(2, 0, 3, 1).reshape(S, BPC, T)
        )
    return part


def _host_backtrack(part, feats, mask, trans):
    """Backpointer recompute + backtrack, bit-exact vs the jax reference."""
    f32 = np.float32
    lengths = mask.astype(np.int64).sum(axis=1)          # (B,)
    last_pos = lengths - 1
    bidx = np.arange(B)

    last_partition = part[last_pos, bidx, :]             # (B, T)
    last_vals = last_partition + trans[:, STOP_TAG][None, :].astype(f32)
    pointer0 = np.argmax(last_vals, axis=1).astype(np.int32)

    decode = np.zeros((S, B), dtype=np.int32)
    decode[S - 1] = pointer0
    ptr = pointer0
    trans_T = np.ascontiguousarray(trans.T)              # trans_T[j, i] = trans[i, j]
    for t in range(S - 2, -1, -1):
        jstar = ptr                                       # decode[t+1]
        fcol = feats[bidx, t + 1, jstar].astype(f32)      # (B,)
        cur = (fcol[:, None] + trans_T[jstar]) + part[t]  # (B, T) f32
        bp_val = np.argmax(cur, axis=1).astype(np.int32)
        new_ptr = np.where(
            t == last_pos, pointer0,
            np.where(t >= lengths, 0, bp_val)
        ).astype(np.int32)
        decode[t] = new_ptr
        ptr = new_ptr
    return decode.T                                       # (B, S)


def kernel(feats, mask, tags, transitions):
    feats = np.asarray(feats, dtype=np.float32)
    mask = np.asarray(mask)
    trans = np.asarray(transitions, dtype=np.float32)
    lengths = mask.astype(np.int64).sum(axis=1)
    perm, glens = _plan_batches(lengths)
    feats_p = np.ascontiguousarray(feats[perm])
    mask_p = np.ascontiguousarray(mask[perm])
    part = _forward_on_device(feats_p, trans, glens)
    decode_p = _host_backtrack(part, feats_p, mask_p, trans)
    out = np.empty_like(decode_p)
    out[perm] = decode_p
    return out
